# revision 4
# baseline (speedup 1.0000x reference)
"""Causal multi-head self-attention on 8 trn2 NeuronCores.

Sharding: core c = (batch b = c//2, head-group g = c%2). Each core handles one
batch element and 6 of the 12 heads: QKV projection for its 384 output dims,
causal attention for its 6 heads, and a partial output projection against the
matching 384 columns of o_proj. Host sums the two partials per batch.

Device-side layout (per core):
  xT  [768, 2048]   x transposed (host-side), d on partitions
  QT/KT pair tiles [128, 2048]: partitions = (head 2p | head 2p+1) x dk=64,
    free dim = sequence. Produced by out = wT.T @ xT matmuls.
  S^T tiles [k, q]: scores transposed, computed with head-pair row tiling
    (K=dk=64 per head, two heads in array rows 0-63 / 64-127).
  exp on ScalarE (PSUM -> SBUF, bf16), causal mask applied as a 0/1 multiply
    on the diagonal blocks only.
  V_aug [k, 65]: V for one head + ones column; A@V matmul then yields both
    O^T (rows 0..63) and the softmax denominator (row 64) in one chain.
  Normalization: denom reciprocal (DVE), broadcast across partitions via a
    K=1 ones matmul on PE, multiply on DVE -> OT tiles [c, s].
  Output projection: out = OT.T @ owT accumulated over the 3 c-blocks.
"""

import numpy as np
import ml_dtypes

B, S, D = 4, 2048, 768
H, DK = 12, 64
NCORES = 8
GH = 6        # heads per core
GO = GH * DK  # 384, per-core slice of the qkv output dim
NP = 3        # head pairs per core
NSB = S // 128   # 16 sequence blocks of 128
NJ = S // 512    # 4 q-chunks of 512

BF16 = ml_dtypes.bfloat16

_CACHE = {}


def _build_bass():
    import concourse.bass as bass  # noqa: F401
    import concourse.tile as tile
    from concourse import bacc, mybir
    from contextlib import ExitStack

    f32 = mybir.dt.float32
    bf16 = mybir.dt.bfloat16
    AF = mybir.ActivationFunctionType

    nc = bacc.Bacc("TRN2", target_bir_lowering=False, debug=False,
                   num_devices=NCORES)

    xT_d = nc.dram_tensor("xT", [D, S], bf16, kind="ExternalInput").ap()
    wqT_d = nc.dram_tensor("wqT", [D, GO], bf16, kind="ExternalInput").ap()
    wkT_d = nc.dram_tensor("wkT", [D, GO], bf16, kind="ExternalInput").ap()
    wvT_d = nc.dram_tensor("wvT", [D, GO], bf16, kind="ExternalInput").ap()
    owT_d = nc.dram_tensor("owT", [GO, D], bf16, kind="ExternalInput").ap()
    mk_d = nc.dram_tensor("mk", [4 * 128, 512], bf16, kind="ExternalInput").ap()
    part_d = nc.dram_tensor("part", [S, D], f32, kind="ExternalOutput").ap()

    ND = D // 128  # 6 d-blocks

    with tile.TileContext(nc) as tc, ExitStack() as ctx:
        pers = ctx.enter_context(tc.tile_pool(name="pers", bufs=1))

        # ---- persistent SBUF tiles -------------------------------------
        xT = [pers.tile([128, S], bf16, tag=f"xT{d}", name=f"xT{d}") for d in range(ND)]
        wq = [pers.tile([128, GO], bf16, tag=f"wq{d}", name=f"wq{d}") for d in range(ND)]
        wk = [pers.tile([128, GO], bf16, tag=f"wk{d}", name=f"wk{d}") for d in range(ND)]
        wv = [pers.tile([128, GO], bf16, tag=f"wv{d}", name=f"wv{d}") for d in range(ND)]
        ow = [pers.tile([128, D], bf16, tag=f"ow{c}", name=f"ow{c}") for c in range(NP)]
        mkt = [pers.tile([128, 512], bf16, tag=f"mk{v}", name=f"mk{v}") for v in range(4)]
        QT = [pers.tile([128, S], bf16, tag=f"QT{p}", name=f"QT{p}") for p in range(NP)]
        KT = [pers.tile([128, S], bf16, tag=f"KT{p}", name=f"KT{p}") for p in range(NP)]
        OT = [pers.tile([128, S], bf16, tag=f"OT{p}", name=f"OT{p}") for p in range(NP)]
        vaug = [[pers.tile([128, 65], bf16, tag=f"va{h}_{kb}", name=f"va{h}_{kb}")
                 for kb in range(NSB)] for h in range(GH)]
        ones_c = pers.tile([1, 64], f32, tag="ones_c")
        nc.vector.memset(ones_c[:], 1.0)

        for d in range(ND):
            nc.sync.dma_start(xT[d][:], xT_d[d * 128:(d + 1) * 128, :])
            nc.sync.dma_start(wq[d][:], wqT_d[d * 128:(d + 1) * 128, :])
            nc.sync.dma_start(wk[d][:], wkT_d[d * 128:(d + 1) * 128, :])
            nc.sync.dma_start(wv[d][:], wvT_d[d * 128:(d + 1) * 128, :])
        for c in range(NP):
            nc.sync.dma_start(ow[c][:], owT_d[c * 128:(c + 1) * 128, :])
        for v in range(4):
            nc.sync.dma_start(mkt[v][:], mk_d[v * 128:(v + 1) * 128, :])

        # ---- QKV projection --------------------------------------------
        with tc.tile_pool(name="pj", space="PSUM", bufs=2) as pj:
            for wsrc, dst in ((wq, QT), (wk, KT)):
                for ob in range(NP):
                    for j in range(NJ):
                        ps = pj.tile([128, 512], f32, tag="pjq")
                        for d in range(ND):
                            nc.tensor.matmul(
                                ps[:],
                                wsrc[d][:, ob * 128:(ob + 1) * 128],
                                xT[d][:, j * 512:(j + 1) * 512],
                                start=(d == 0), stop=(d == ND - 1))
                        nc.vector.tensor_copy(
                            dst[ob][:, j * 512:(j + 1) * 512], ps[:])
            for sb in range(NSB):
                ps = pj.tile([128, GO], f32, tag="pjv")
                for d in range(ND):
                    nc.tensor.matmul(
                        ps[:],
                        xT[d][:, sb * 128:(sb + 1) * 128],
                        wv[d][:],
                        start=(d == 0), stop=(d == ND - 1))
                for h in range(GH):
                    nc.vector.tensor_copy(
                        vaug[h][sb][:, 0:64], ps[:, h * 64:(h + 1) * 64])
                    nc.vector.memset(vaug[h][sb][:, 64:65], 1.0)

        # ---- attention --------------------------------------------------
        with tc.tile_pool(name="spp", space="PSUM", bufs=1) as spp, \
             tc.tile_pool(name="avp", space="PSUM", bufs=2) as avp, \
             tc.tile_pool(name="bcp", space="PSUM", bufs=2) as bcp, \
             tc.tile_pool(name="exp", bufs=9) as expp, \
             tc.tile_pool(name="sml", bufs=3) as sml:
            for p in range(NP):
                for j in range(NJ):
                    nkb = 4 * j + 4
                    exref = {}
                    for g in range(0, nkb, 2):
                        sp = spp.tile([128, 2048], f32, tag="sp")
                        segs = [(0, g), (1, g), (0, g + 1), (1, g + 1)]
                        for si, (hh, kb) in enumerate(segs):
                            nc.tensor.matmul(
                                sp[:, si * 512:(si + 1) * 512],
                                KT[p][hh * 64:(hh + 1) * 64,
                                      kb * 128:(kb + 1) * 128],
                                QT[p][hh * 64:(hh + 1) * 64,
                                      j * 512:(j + 1) * 512],
                                start=True, stop=True,
                                tile_position=(hh * 64, 0))
                        ex = expp.tile([128, 2048], bf16, tag="ex")
                        nc.scalar.activation(ex[:], sp[:], AF.Exp)
                        for si, (hh, kb) in enumerate(segs):
                            sl = ex[:, si * 512:(si + 1) * 512]
                            if kb >= 4 * j:  # diagonal block: causal mask
                                nc.vector.tensor_mul(sl, sl,
                                                     mkt[kb - 4 * j][:])
                            exref[(hh, kb)] = sl
                    for hh in range(2):
                        h = 2 * p + hh
                        av = avp.tile([65, 512], f32, tag="av")
                        for kb in range(nkb):
                            nc.tensor.matmul(
                                av[:], vaug[h][kb][:], exref[(hh, kb)],
                                start=(kb == 0), stop=(kb == nkb - 1))
                        rcp = sml.tile([1, 512], f32, tag="rcp")
                        nc.vector.reciprocal(rcp[:], av[64:65, :])
                        bc = bcp.tile([64, 512], f32, tag="bc")
                        nc.tensor.matmul(bc[:], ones_c[:], rcp[:],
                                         start=True, stop=True)
                        bcs = sml.tile([64, 512], f32, tag="bcs")
                        nc.vector.tensor_copy(bcs[:], bc[:])
                        nc.vector.tensor_mul(
                            OT[p][hh * 64:(hh + 1) * 64,
                                  j * 512:(j + 1) * 512],
                            av[0:64, :], bcs[:])

        # ---- output projection -----------------------------------------
        with tc.tile_pool(name="opp", space="PSUM", bufs=2) as opp, \
             tc.tile_pool(name="stg", bufs=3) as stg:
            for sb in range(NSB):
                op = opp.tile([128, D], f32, tag="op")
                for cb in range(NP):
                    lhs = OT[cb][:, sb * 128:(sb + 1) * 128]
                    nc.tensor.matmul(op[:, 0:512], lhs, ow[cb][:, 0:512],
                                     start=(cb == 0), stop=(cb == NP - 1))
                    nc.tensor.matmul(op[:, 512:768], lhs, ow[cb][:, 512:768],
                                     start=(cb == 0), stop=(cb == NP - 1))
                st = stg.tile([128, D], f32, tag="st")
                nc.vector.tensor_copy(st[:], op[:])
                nc.sync.dma_start(part_d[sb * 128:(sb + 1) * 128, :], st[:])

    nc.compile()
    return nc


def _prep_in_maps(in_features, qkv_proj_weight, o_proj_weight):
    """Per-core input dict (host-side shard + transpose + cast)."""
    # causal 0/1 masks for the four diagonal block offsets
    r = np.arange(128)[:, None]
    c = np.arange(512)[None, :]
    mk = np.concatenate(
        [(r <= c - 128 * v).astype(np.float32) for v in range(4)], axis=0)
    mk = mk.astype(BF16)

    scale = 1.0 / np.sqrt(np.float32(DK))
    in_maps = []
    for core in range(NCORES):
        b, g = core // 2, core % 2
        sl = slice(g * GO, (g + 1) * GO)
        xT = np.ascontiguousarray(in_features[b].T).astype(BF16)
        wqT = np.ascontiguousarray((qkv_proj_weight[0][sl, :] * scale).T
                                   ).astype(BF16)
        wkT = np.ascontiguousarray(qkv_proj_weight[1][sl, :].T).astype(BF16)
        wvT = np.ascontiguousarray(qkv_proj_weight[2][sl, :].T).astype(BF16)
        owT = np.ascontiguousarray(o_proj_weight[:, sl].T).astype(BF16)
        in_maps.append({"xT": xT, "wqT": wqT, "wkT": wkT, "wvT": wvT,
                        "owT": owT, "mk": mk})
    return in_maps


def _get_runner():
    """Persistent sharded-jit runner over the 8 NeuronCores.

    Mirrors bass_utils.run_bass_kernel_spmd's axon path
    (bass2jax.run_bass_via_pjrt), but keeps the jitted executable cached
    across calls and skips install_neuronx_cc_hook: under axon the
    bass_exec custom-call is compiled terminal-side, and the client-side
    hook rejects the SPMD-partitioned HLO.
    """
    if "runner" in _CACHE:
        return _CACHE["runner"]

    import jax
    from concourse import mybir
    from concourse.bass2jax import _bass_exec_p, partition_id_tensor
    from jax.sharding import Mesh, PartitionSpec
    from jax.experimental.shard_map import shard_map

    nc = _build_bass()

    partition_name = (nc.partition_id_tensor.name
                      if nc.partition_id_tensor else None)
    in_names, out_names, out_avals, zero_outs = [], [], [], []
    for alloc in nc.m.functions[0].allocations:
        if not isinstance(alloc, mybir.MemoryLocationSet):
            continue
        name = alloc.memorylocations[0].name
        if alloc.kind == "ExternalInput":
            if name != partition_name:
                in_names.append(name)
        elif alloc.kind == "ExternalOutput":
            out_names.append(name)
            shape = tuple(alloc.tensor_shape)
            dtype = mybir.dt.np(alloc.dtype)
            out_avals.append(jax.core.ShapedArray(shape, dtype))
            zero_outs.append(np.zeros(shape, dtype))
    n_params = len(in_names)
    n_outs = len(out_avals)
    all_in = list(in_names) + out_names + (
        [partition_name] if partition_name else [])

    def _body(*args):
        operands = list(args)
        if partition_name is not None:
            operands.append(partition_id_tensor())
        return tuple(_bass_exec_p.bind(
            *operands,
            out_avals=tuple(out_avals),
            in_names=tuple(all_in),
            out_names=tuple(out_names),
            lowering_input_output_aliases=(),
            sim_require_finite=True, sim_require_nnan=True, nc=nc))

    devices = jax.devices()[:NCORES]
    mesh = Mesh(np.asarray(devices), ("core",))
    fn = jax.jit(
        shard_map(_body, mesh=mesh,
                  in_specs=(PartitionSpec("core"),) * (n_params + n_outs),
                  out_specs=(PartitionSpec("core"),) * n_outs,
                  check_rep=False),
        donate_argnums=tuple(range(n_params, n_params + n_outs)),
        keep_unused=True)

    def run(in_maps):
        per_core = [[np.asarray(m[n]) for n in in_names] for m in in_maps]
        concat_in = [np.concatenate([per_core[c][i] for c in range(NCORES)],
                                    axis=0) for i in range(n_params)]
        concat_zeros = [np.zeros((NCORES * z.shape[0], *z.shape[1:]), z.dtype)
                        for z in zero_outs]
        out_arrs = fn(*concat_in, *concat_zeros)
        return np.asarray(out_arrs[out_names.index("part")]).reshape(
            NCORES, S, D)

    _CACHE["nc"] = nc
    _CACHE["runner"] = run
    return run


def kernel(in_features, qkv_proj_weight, o_proj_weight):
    run = _get_runner()
    in_maps = _prep_in_maps(np.asarray(in_features, np.float32),
                            np.asarray(qkv_proj_weight, np.float32),
                            np.asarray(o_proj_weight, np.float32))
    parts = run(in_maps)
    out = np.empty((B, S, D), np.float32)
    for b in range(B):
        out[b] = parts[2 * b] + parts[2 * b + 1]
    return out
